# revision 74
# baseline (speedup 1.0000x reference)
"""Trainium2 Bass kernel for nn_Attention_54159537603130.

Dense GQA attention block (QKV proj + RoPE + causal attention + out proj),
sharded over 8 NeuronCores as (batch=2) x (kv-head groups=4).  Each core
computes a [S, DIM] partial of the output projection (wo is row-sharded);
the host sums the 4 group partials per batch.

All on-chip matmul operands live in "transposed" feature-on-partition
layouts so no large on-chip transposes are needed:
  Q^T/K^T [d, t]  -> scores^T tiles [t, s] directly
  V token-major [t, d] -> out^T = V^T @ P^T via PE accumulation
  out^T [d, s] is exactly the lhsT of the wo matmul.

Softmax runs without max-subtraction (logits are O(10) here).  The
1/sqrt(HD) scale is folded into the exp activation's scale.

Phase-2 (causal) highlights:
  - causal masking is a post-exp multiply by a shared [128,128] 0/1
    triangle (numerically identical to adding -1e9 pre-exp, off the
    scores->exp critical path, DVE 16-bit fast path);
  - diagonal tiles only process their valid column suffix, ordered so
    psum accumulate start/stop flags stay full-width;
  - scores for two i-chunks land in one [128,2,512] two-bank psum tile
    and are exponentiated by ONE ACT instruction (the ~260ns fixed ACT
    cost per instruction would otherwise dominate);
  - row sums are [128,128]-config selector matmuls (ones in column 32j)
    so the PE never switches tile config (an M=1 ones-vector rowsum
    costs ~2x95ns of reconfig per i-chunk), accumulated across a whole
    head into rows 0/32/64/96 of one psum bank; ln/exp then runs once
    per head over the full tile (ACT cost is free-size only);
  - a two-stage software pipeline (scores -> exp+mask -> PV+rowsum)
    keeps the PE fed past the ACT exp and DVE mask latency;
  - the reciprocal broadcast matmul + in-place normalize of out^T are
    deferred into the next head's stream so the single psb bank's WAR
    chain never stalls the PE.
RoPE's (2i,2i+1) partition swap runs as two stride-2 partition DMAs
instead of a PE permute matmul.  Token quarters are projected in pairs
sharing each weight-chunk DMA, with latency-critical transfers sliced
across the 8 parallel DMA queues (a whole-tile DMA only gets 1/8 of
the HBM bandwidth when other transfers are in flight).
"""

import os
import sys

sys.path.insert(0, "/opt/trn_rl_repo")

import numpy as np
import ml_dtypes

import concourse.bass as bass
import concourse.tile as tile
from concourse import mybir

BF16 = mybir.dt.bfloat16
F32 = mybir.dt.float32
NPBF16 = ml_dtypes.bfloat16

DIM, NH, NKV, HD = 4096, 32, 8, 128
B, S = 2, 2048
NCORES = 8
GQ = 8  # q heads per core
GKV = 2  # kv heads per core
MQ = GQ * HD  # 1024 q-proj cols per core
MKV = GKV * HD  # 256 kv-proj cols per core
SC = 1.0 / np.sqrt(HD)
NEG_INF = -1e9

LAST_EXEC_TIME_NS = None
LAST_RESULTS = None


def _install_ntff_hook():
    """antenv.axon_hooks is absent in this image; reconstruct the NTFF
    profiling hook via ctypes against libaxon_pjrt.so (only used when
    BASS_TRACE is set)."""
    import types
    import contextlib
    import ctypes

    if "antenv.axon_hooks" in sys.modules:
        return
    try:
        lib = ctypes.CDLL("/opt/axon/libaxon_pjrt.so")
        have = hasattr(lib, "axon_start_nrt_profile")
    except OSError:
        have = False

    if have:
        lib.axon_start_nrt_profile.argtypes = [
            ctypes.POINTER(ctypes.c_int64),
            ctypes.c_size_t,
        ]
        lib.axon_start_nrt_profile.restype = ctypes.c_int64
        lib.axon_stop_nrt_profile.argtypes = [ctypes.c_char_p]
        lib.axon_stop_nrt_profile.restype = ctypes.c_int64

        @contextlib.contextmanager
        def _hook(output_dir, device_ids):
            import jax

            jax.devices()
            if device_ids:
                ids = (ctypes.c_int64 * len(device_ids))(*device_ids)
                rc = lib.axon_start_nrt_profile(ids, len(device_ids))
            else:
                rc = lib.axon_start_nrt_profile(None, 0)
            if rc != 0:
                raise RuntimeError(f"axon_start_nrt_profile rc={rc}")
            try:
                yield
            finally:
                n = lib.axon_stop_nrt_profile(str(output_dir).encode())
                print(f"profile: {n} file(s) written to {output_dir}")

        hook = _hook
    else:
        hook = None

    mod = types.ModuleType("antenv.axon_hooks")
    mod.get_axon_ntff_profile_hook = lambda: hook
    mod.set_axon_ntff_profile_hook = lambda h: None
    sys.modules["antenv.axon_hooks"] = mod


def split_excess_waits(nc, max_waits=1):
    """walrus codegen supports very few sync waits per instruction while
    Tile's tail/release drains can carry several; hoist excess onto NOPs."""
    for fn in nc.m.functions:
        for blk in fn.blocks:
            insts = blk.instructions
            changed = False
            i = 0
            while i < len(insts):
                inst = insts[i]
                si = inst.sync_info
                if (
                    si is not None
                    and si.on_wait is not None
                    and len(si.on_wait) > max_waits
                ):
                    w = si.on_wait
                    k = 0
                    while len(w) > max_waits:
                        nop = mybir.InstNoOp(
                            name=f"{inst.name}_wsplit{k}",
                            engine=inst.engine,
                            ins=[],
                            outs=[],
                        )
                        nop.sync_info = mybir.SyncInfo(
                            on_wait=w[:max_waits], on_update=[]
                        )
                        insts.insert(i, nop)
                        i += 1
                        w = w[max_waits:]
                        k += 1
                    inst.sync_info = mybir.SyncInfo(on_wait=w, on_update=si.on_update)
                    changed = True
                i += 1
            if changed:
                blk.instructions = insts


def _build(causal: bool):
    nc = bass.Bass("TRN2", target_bir_lowering=False, debug=False)
    Exp = mybir.ActivationFunctionType.Exp
    Ln = mybir.ActivationFunctionType.Ln

    # DRAM I/O — all inputs pre-tiled on the host into SBUF-friendly
    # [partition, ...] layouts with large contiguous per-partition runs.
    xq_d = nc.dram_tensor("xq", [4, 128, 32, 512], BF16, kind="ExternalInput").ap()
    wq_d = nc.dram_tensor("wq", [8, 128, 32, 128], BF16, kind="ExternalInput").ap()
    wk_d = nc.dram_tensor("wk", [2, 128, 32, 128], BF16, kind="ExternalInput").ap()
    wv_d = nc.dram_tensor("wv", [128, 32, 256], BF16, kind="ExternalInput").ap()
    wo_d = nc.dram_tensor("wo", [128, 8, 4096], BF16, kind="ExternalInput").ap()
    ce_d = nc.dram_tensor("ce", [128, 2048], BF16, kind="ExternalInput").ap()
    s2_d = nc.dram_tensor("s2", [128, 2048], BF16, kind="ExternalInput").ap()
    sel_d = nc.dram_tensor("seld", [128, 4, 128], BF16, kind="ExternalInput").ap()
    if causal:
        # one shared [128,128] 0/1 triangle masks every diagonal tile
        tri_d = nc.dram_tensor("trid", [128, 128], BF16, kind="ExternalInput").ap()
    else:
        mask_d = nc.dram_tensor(
            "maskt", [128, 16, 2048], BF16, kind="ExternalInput"
        ).ap()
    out_d = nc.dram_tensor("out", [128, 16, 4096], BF16, kind="ExternalOutput").ap()

    with tile.TileContext(nc) as tc:
        with (
            tc.tile_pool(name="consts", bufs=1) as consts,
            tc.tile_pool(name="persist", bufs=1) as persist,
        ):
            # tiles declared up front; DMA emission ordered so the first
            # projection's deps (x quarter 0, wq chunk 0) load first.
            ce_t = consts.tile([128, 2048], BF16)
            s2_t = consts.tile([128, 2048], BF16)
            sel_t = consts.tile([128, 4, 128], BF16)
            ones_col = consts.tile([128, 1], BF16)
            ones_row = consts.tile([1, 128], BF16)
            ones128 = consts.tile([128, 128], BF16)

            qrot = persist.tile([128, GQ, 2048], BF16)
            krot = persist.tile([128, GKV, 2048], BF16)
            vtok = persist.tile([128, 16, MKV], BF16)
            if causal:
                tri_t = persist.tile([128, 128], BF16)

            # ---------------- Phase 1: QKV projections + RoPE --------------
            with (
                tc.tile_pool(name="p1", bufs=1) as p1,
                tc.tile_pool(name="p1ps", bufs=1, space="PSUM") as pps,
            ):
                wv_t = p1.tile([128, 32, 256], BF16, tag="wv", bufs=1)

                rope_tail = []

                def rope(ps, dst, toff, defer=False):
                    # dst = ce*q + s2*pairswap(q) over a [128,1024] token
                    # pair (both quarters of one head at once: one psum
                    # read window instead of two).  The (2i,2i+1) partition
                    # swap runs as two stride-2 partition DMAs (sbuf->sbuf)
                    # instead of a PE permute matmul, and the multiplies
                    # stay all-bf16-SBUF for the DVE fast path; the second
                    # product adds into dst in place.  Only the psum->sbuf
                    # copy is immediate (it frees the psum slot); during
                    # the DMA-critical first token pair the swap DMAs are
                    # deferred behind the next weight chunk's DMA.
                    qb = p1.tile([128, 1024], BF16, tag="ropeb", bufs=3)
                    nc.scalar.copy(out=qb, in_=ps)

                    def tail():
                        sw = p1.tile([128, 1024], BF16, tag="swap", bufs=2)
                        nc.gpsimd.dma_start(out=sw[0:127:2, :], in_=qb[1:128:2, :])
                        nc.gpsimd.dma_start(out=sw[1:128:2, :], in_=qb[0:127:2, :])
                        nc.vector.tensor_mul(dst, qb, ce_t[:, toff : toff + 1024])
                        bt = p1.tile([128, 1024], BF16, tag="ropec", bufs=2)
                        nc.vector.tensor_mul(bt, sw, s2_t[:, toff : toff + 1024])
                        nc.vector.tensor_add(dst, dst, bt)

                    if defer:
                        rope_tail.append(tail)
                    else:
                        tail()

                def flush_ropes():
                    while rope_tail:
                        rope_tail.pop(0)()

                # token quarters processed in pairs sharing each weight
                # chunk DMA (halves the wq/wk streaming traffic, which
                # otherwise saturates the DMA queue early in the kernel)
                xh_t = {}

                def load_xh(q):
                    xh = p1.tile([128, 32, 512], BF16, tag="xh", bufs=3, name="xh")
                    xh_t[q] = xh
                    if q == 0:
                        # interleave fine-grained slices of the first x/wq
                        # tiles so the first matmuls start ~4MB sooner, with
                        # the rope constants right behind
                        # transfers complete in issue order at the full
                        # aggregate DMA rate (packets spray across all 16
                        # engines), so issue in DEMAND order: the first
                        # matmul starts once wqc0's and xh's first slices
                        # land and then races the rest of the stream
                        nc.gpsimd.dma_start(out=wqc0[:, 0:16], in_=wq_d[0, :, 0:16])
                        nc.gpsimd.dma_start(out=xh[:, 0:8], in_=xq_d[q, :, 0:8])
                        nc.gpsimd.dma_start(out=xh[:, 8:16], in_=xq_d[q, :, 8:16])
                        nc.gpsimd.dma_start(out=wqc0[:, 16:32], in_=wq_d[0, :, 16:32])
                        nc.gpsimd.dma_start(out=xh[:, 16:24], in_=xq_d[q, :, 16:24])
                        nc.gpsimd.dma_start(out=xh[:, 24:32], in_=xq_d[q, :, 24:32])
                        nc.vector.memset(ones_col, 1.0)
                        nc.vector.memset(ones_row, 1.0)
                        nc.vector.memset(ones128, 1.0)
                    elif q == 1:
                        for c in range(0, 32, 8):
                            nc.gpsimd.dma_start(
                                out=xh[:, c : c + 8], in_=xq_d[q, :, c : c + 8]
                            )
                    else:
                        nc.gpsimd.dma_start(out=xh[:, 0:16], in_=xq_d[q, :, 0:16])
                        nc.gpsimd.dma_start(out=xh[:, 16:32], in_=xq_d[q, :, 16:32])

                wqc0 = p1.tile([128, 32, 128], BF16, tag="wc", bufs=3)
                load_xh(0)
                load_xh(1)
                # rope constants ride behind x quarter 1: they are only
                # needed by the off-critical-path rope DVE multiplies
                nc.gpsimd.dma_start(out=ce_t, in_=ce_d)
                nc.gpsimd.dma_start(out=s2_t, in_=s2_d)

                for qp in (0, 2):
                    if qp == 2:
                        load_xh(3)
                        if causal:
                            nc.gpsimd.dma_start(out=tri_t, in_=tri_d)
                            nc.gpsimd.dma_start(out=sel_t, in_=sel_d)
                    for m in range(GQ):
                        if qp == 0 and m == 0:
                            wqc = wqc0
                        else:
                            wqc = p1.tile([128, 32, 128], BF16, tag="wc", bufs=3)
                            nc.gpsimd.dma_start(
                                out=wqc[:, 0:16], in_=wq_d[m, :, 0:16]
                            )
                            nc.gpsimd.dma_start(
                                out=wqc[:, 16:32], in_=wq_d[m, :, 16:32]
                            )
                        if qp == 0 and m == 2:
                            nc.gpsimd.dma_start(out=wv_t, in_=wv_d)
                        if qp == 0 and m == 5:
                            load_xh(2)
                        flush_ropes()
                        t0 = 512 * qp
                        ps = pps.tile([128, 1024], F32, tag="proj", bufs=2)
                        for d in range(32):
                            for half in (0, 1):
                                nc.tensor.matmul(
                                    out=ps[:, 512 * half : 512 * half + 512],
                                    lhsT=wqc[:, d],
                                    rhs=xh_t[qp + half][:, d],
                                    start=(d == 0),
                                    stop=(d == 31),
                                )
                        rope(ps, qrot[:, m, t0 : t0 + 1024], t0,
                             defer=(qp == 0 and m < 6))
                    for m in range(GKV):
                        wkc = p1.tile([128, 32, 128], BF16, tag="wc", bufs=3)
                        nc.gpsimd.dma_start(out=wkc[:, 0:16], in_=wk_d[m, :, 0:16])
                        nc.gpsimd.dma_start(out=wkc[:, 16:32], in_=wk_d[m, :, 16:32])
                        flush_ropes()
                        t0 = 512 * qp
                        ps = pps.tile([128, 1024], F32, tag="proj", bufs=2)
                        for d in range(32):
                            for half in (0, 1):
                                nc.tensor.matmul(
                                    out=ps[:, 512 * half : 512 * half + 512],
                                    lhsT=wkc[:, d],
                                    rhs=xh_t[qp + half][:, d],
                                    start=(d == 0),
                                    stop=(d == 31),
                                )
                        rope(ps, krot[:, m, t0 : t0 + 1024], t0)
                    for q in (qp, qp + 1):
                        for tv in range(4):
                            psv = pps.tile([128, 256], F32, tag="vproj", bufs=3)
                            for d in range(32):
                                nc.tensor.matmul(
                                    out=psv,
                                    lhsT=xh_t[q][:, d, 128 * tv : 128 * tv + 128],
                                    rhs=wv_t[:, d],
                                    start=(d == 0),
                                    stop=(d == 31),
                                )
                            nc.scalar.copy(out=vtok[:, 4 * q + tv, :], in_=psv)

            # outT lives from phase 2 through phase 3; allocated after the
            # phase-1 pools release so SBUF peaks stay under budget. wo is
            # prefetched here so its 8MB load overlaps phase 2.
            outT_pool = tc.alloc_tile_pool(name="po", bufs=1)
            outT = outT_pool.tile([128, GQ, 2048], BF16)
            wo_pool = tc.alloc_tile_pool(name="pwo", bufs=1)
            wo_t = wo_pool.tile([128, 8, 4096], BF16)
            nc.gpsimd.dma_start(out=wo_t, in_=wo_d)

            # ---------------- Phase 2: attention ---------------------------
            with (
                tc.tile_pool(name="p2", bufs=1) as p2,
                tc.tile_pool(name="p2ps", bufs=1, space="PSUM") as pps2,
            ):
                if causal:
                    # Two-stage software pipeline: scores run LAG1 items
                    # ahead of exp+mask, which run one more item ahead of
                    # PV+rowsum, so neither the ACT exp latency nor the DVE
                    # mask multiply ever gates the PE.
                    # PSUM: pss 3 + pso 2 + rs 2 + psb 1 = 8 banks.
                    # Diagonal tiles of the j>=1 blocks only process their
                    # valid column suffix [c0:512); they are ordered first
                    # in each block so the start flag (p=0, full width) and
                    # stop flag (last non-diag, full width) cover the whole
                    # tile.  Rowsums use a [128,128] selector lhsT (ones in
                    # column 32j) so every phase-2 matmul runs in the same
                    # 128x128 PE tile config — an M=1 ones-vector rowsum
                    # switches the PE to a 128x32 config and back, costing
                    # ~95ns twice per i-chunk.  One accumulate group spans
                    # the whole head (start wipes the bank at head start;
                    # other rows just accumulate selector zeros), and the
                    # banks alternate per head parity so the next head's
                    # rowsums never wait on this head's ln read.
                    rs_a = pps2.tile([128, 512], F32, name="rs_a")
                    rs_par = [rs_a, rs_a]
                    LAG1 = 1

                    # i-chunk pairs: one two-bank psum tile and ONE exp
                    # instruction per pair (the ~260ns fixed ACT cost per
                    # instruction is a big slice of phase 2).  Diagonal
                    # tiles are interleaved with full-width ones so the
                    # paired exp's [cmin:512) window wastes few columns.
                    pairs = []  # item = (h,j,i,c0,first,last,rs_start,rs_stop)
                    for h in range(GQ):
                        for bi, j in enumerate(range(4)):
                            if j == 0:
                                ilist = [(i, 128 * i) for i in range(4)]
                            else:
                                dg = [(4 * j + p, 128 * p) for p in range(4)]
                                nd = [(i, 0) for i in range(4 * j)]
                                ilist = [dg[0], nd[0], dg[1], dg[2], dg[3], nd[1]]
                                ilist += nd[2:]
                            n = len(ilist)
                            items = [
                                (h, j, i, c0, idx == 0, idx == n - 1,
                                 bi == 0 and idx == 0, bi == 3 and idx == n - 1)
                                for idx, (i, c0) in enumerate(ilist)
                            ]
                            for t in range(0, len(items), 2):
                                pairs.append((items[t], items[t + 1]))

                    def emit_scores(pair):
                        pss = pps2.tile([128, 2, 512], F32, tag="pss", bufs=2)
                        for half, item in enumerate(pair):
                            h, j, i, c0, first, last, rs_s, rs_e = item
                            kv = h // 4
                            s0 = 512 * j
                            nc.tensor.matmul(
                                out=pss[:, half, c0:512],
                                lhsT=krot[:, kv, 128 * i : 128 * i + 128],
                                rhs=qrot[:, h, s0 + c0 : s0 + 512],
                                start=True,
                                stop=True,
                            )
                        return pss

                    def emit_expmask(pair, pss):
                        cmin = min(pair[0][3], pair[1][3])
                        e = p2.tile([128, 2, 512], BF16, tag="exp", bufs=3)
                        nc.scalar.activation(
                            out=e[:, :, cmin:512],
                            in_=pss[:, :, cmin:512],
                            func=Exp,
                            scale=SC,
                        )
                        for half, item in enumerate(pair):
                            h, j, i, c0, first, last, rs_s, rs_e = item
                            if i >= 4 * j:
                                sl = e[:, half, c0 : c0 + 128]
                                nc.vector.tensor_mul(sl, sl, tri_t)
                        return e

                    def emit_pvrs(pair, e):
                        for half, item in enumerate(pair):
                            h, j, i, c0, first, last, rs_s, rs_e = item
                            kv = h // 4
                            s0 = 512 * j
                            rs_t = rs_par[h % 2]
                            pso = pso_cur[0]
                            nc.tensor.matmul(
                                out=pso[:, c0:512],
                                lhsT=vtok[:, i, 128 * kv : 128 * kv + 128],
                                rhs=e[:, half, c0:512],
                                start=first,
                                stop=last,
                            )
                            nc.tensor.matmul(
                                out=rs_t[:, c0:512],
                                lhsT=sel_t[:, j, :],
                                rhs=e[:, half, c0:512],
                                start=rs_s,
                                stop=rs_e,
                            )
                            if last:
                                # block done: park the raw out^T
                                nc.vector.tensor_copy(
                                    out=outT[:, h, s0 : s0 + 512], in_=pso
                                )
                                pso_cur[0] = pps2.tile(
                                    [128, 512], F32, tag="pso", bufs=2, name="pso_n"
                                )
                                if rs_e:
                                    finalize_h(h)

                    def finalize_h(h):
                        # batched softmax denominators for this head's 4
                        # blocks (rows 0/32/64/96), computed on the DVE
                        # (idle here) instead of ACT ln+exp: the ACT queue
                        # sits 2+ paired exps deep at head boundaries, so
                        # an ACT-produced recip stalls the psb matmul.
                        # The per-j psum broadcast + in-place multiply are
                        # deferred >=2 steps into the next head's stream so
                        # neither the recip nor the single psb bank's WAR
                        # chain ever gates the PE.
                        lnr = p2.tile([128, 512], F32, tag="lnr", bufs=2)
                        nc.scalar.activation(out=lnr, in_=rs_par[h % 2], func=Ln)
                        recip = p2.tile([128, 512], BF16, tag="recip", bufs=2)
                        nc.scalar.activation(out=recip, in_=lnr, func=Exp, scale=-1.0)
                        for j in range(4):
                            deferred.append((step2[0], h, j, recip))

                    def run_deferred(drain=False):
                        rdy, h, j, recip = deferred.pop(0)
                        if drain and j % 2 == 1:
                            pbig = pps2.tile(
                                [128, 2, 512], F32, tag="pss", bufs=2, name="psbd"
                            )
                            psb = pbig[:, 0, :]
                        else:
                            psb = pps2.tile([128, 512], F32, tag="psb", bufs=1)
                        nc.tensor.matmul(
                            out=psb,
                            lhsT=ones128[32 * j : 32 * j + 1, :],
                            rhs=recip[32 * j : 32 * j + 1, :],
                            start=True,
                            stop=True,
                            tile_position=(32 * j, 0),
                        )
                        sl = outT[:, h, 512 * j : 512 * j + 512]
                        nc.vector.tensor_mul(sl, sl, psb)

                    pso0 = pps2.tile([128, 512], F32, tag="pso", bufs=2, name="pso_n")
                    pso_cur = [pso0]
                    step2 = [0]
                    deferred = []
                    q_exp = []
                    q_pv = []
                    for k in range(len(pairs) + LAG1 + 1):
                        if k < len(pairs):
                            q_exp.append((pairs[k], emit_scores(pairs[k])))
                        if k >= LAG1 and q_exp:
                            pr, pss = q_exp.pop(0)
                            q_pv.append((pr, emit_expmask(pr, pss)))
                        if k >= LAG1 + 1 and q_pv:
                            pr, e = q_pv.pop(0)
                            step2[0] += 1
                            emit_pvrs(pr, e)
                            if deferred and deferred[0][0] <= step2[0]:
                                run_deferred()
                    while deferred:
                        run_deferred(drain=True)
                else:
                    # generic-mask fallback: original structure (mask added
                    # pre-exp, per-block rowsum + broadcast matmul), j-outer
                    # so the mask streams in [128, 16, 512] column slices.
                    def finalize(fin):
                        pso_, psr_, h_, s0_ = fin
                        nc.scalar.activation(out=psr_, in_=psr_, func=Ln)
                        rp = p2.tile([1, 512], BF16, tag="rp", bufs=2)
                        nc.scalar.activation(out=rp, in_=psr_, func=Exp, scale=-1.0)
                        psb = pps2.tile([128, 512], F32, tag="psb", bufs=1)
                        nc.tensor.matmul(
                            out=psb, lhsT=ones_row, rhs=rp, start=True, stop=True
                        )
                        rb = p2.tile([128, 512], F32, tag="rb", bufs=2)
                        nc.vector.tensor_copy(out=rb, in_=psb)
                        nc.vector.tensor_mul(outT[:, h_, s0_ : s0_ + 512], pso_, rb)

                    pending = None
                    for j in range(4):
                        s0 = 512 * j
                        mask_j = p2.tile([128, 16, 512], BF16, tag="mzk", bufs=2)
                        nc.gpsimd.dma_start(
                            out=mask_j, in_=mask_d[:, :, s0 : s0 + 512]
                        )
                        for h in range(GQ):
                            kv = h // 4
                            n_i = 16
                            pso = pps2.tile([128, 512], F32, tag="pso", bufs=2)
                            psr = pps2.tile([1, 512], F32, tag="psr", bufs=2)
                            for idx in range(n_i):
                                pss = pps2.tile([128, 512], F32, tag="pss", bufs=3)
                                nc.tensor.matmul(
                                    out=pss,
                                    lhsT=krot[:, kv, 128 * idx : 128 * idx + 128],
                                    rhs=qrot[:, h, s0 : s0 + 512],
                                    start=True,
                                    stop=True,
                                )
                                nc.vector.tensor_add(pss, pss, mask_j[:, idx, :])
                                e = p2.tile([128, 512], BF16, tag="exp", bufs=6)
                                nc.scalar.activation(out=e, in_=pss, func=Exp, scale=SC)
                                nc.tensor.matmul(
                                    out=pso,
                                    lhsT=vtok[:, idx, 128 * kv : 128 * kv + 128],
                                    rhs=e,
                                    start=(idx == 0),
                                    stop=(idx == n_i - 1),
                                )
                                nc.tensor.matmul(
                                    out=psr[0:1, :],
                                    lhsT=ones_col,
                                    rhs=e,
                                    start=(idx == 0),
                                    stop=(idx == n_i - 1),
                                )
                                if idx == 0 and pending is not None:
                                    finalize(pending)
                                    pending = None
                            if pending is not None:
                                finalize(pending)
                            pending = (pso, psr, h, s0)
                    finalize(pending)

            # ---------------- Phase 3: output projection -------------------
            with (
                tc.tile_pool(name="p3", bufs=1) as p3,
                tc.tile_pool(name="p3ps", bufs=1, space="PSUM") as pps3,
            ):
                # m-outer ordering: one LDWEIGHTS per 8 matmuls (all 8 PSUM
                # banks accumulate in parallel across the dc dimension)
                for s in range(16):
                    psfs = [
                        pps3.tile(
                            [128, 512], F32, tag="psf", bufs=8, name=f"psf_{s}_{dc}"
                        )
                        for dc in range(8)
                    ]
                    for m in range(8):
                        for dc in range(8):
                            nc.tensor.matmul(
                                out=psfs[dc],
                                lhsT=outT[:, m, 128 * s : 128 * s + 128],
                                rhs=wo_t[:, m, 512 * dc : 512 * dc + 512],
                                start=(m == 0),
                                stop=(m == 7),
                            )
                    for dc in range(8):
                        ot = p3.tile([128, 512], BF16, tag="ot", bufs=16)
                        if dc % 2 == 0:
                            nc.scalar.copy(out=ot, in_=psfs[dc])
                        else:
                            nc.vector.tensor_copy(out=ot, in_=psfs[dc])
                        nc.gpsimd.dma_start(
                            out=out_d[:, s, 512 * dc : 512 * dc + 512], in_=ot
                        )

            wo_pool.release()
            outT_pool.release()

    return nc


def _prep_inputs(x, freqs_cos, freqs_sin, mask, wq, wk, wv, wo, causal):
    """Host-side shard + retile into the DMA layouts declared in _build."""
    f32 = np.float32

    # RoPE planes [128, 2048]: ce[2i,t]=ce[2i+1,t]=cos[t,i];
    # s2[2i,t]=-sin[t,i], s2[2i+1,t]=+sin[t,i].  (1/sqrt(HD) now lives in
    # the exp activation scale, so q and k share one pair of planes.)
    cos_t = np.asarray(freqs_cos, f32).T  # [64, 2048]
    sin_t = np.asarray(freqs_sin, f32).T
    ce = np.repeat(cos_t, 2, axis=0).astype(NPBF16)  # [128, 2048]
    s2 = np.empty((HD, S), f32)
    s2[0::2] = -sin_t
    s2[1::2] = sin_t
    s2 = s2.astype(NPBF16)

    maskT = np.ascontiguousarray(np.asarray(mask, f32).T)  # [t, s]
    if causal:
        tri_host = (
            np.arange(128)[:, None] <= np.arange(128)[None, :]
        ).astype(NPBF16)
        sel_host = np.zeros((128, 4, 128), NPBF16)
        for j in range(4):
            sel_host[:, j, 32 * j] = 1
    else:
        # additive mask on UNSCALED scores: pre-divide by SC
        mask_host = np.ascontiguousarray(
            (maskT / SC).reshape(16, 128, 2048).transpose(1, 0, 2)
        ).astype(NPBF16)

    # per-batch x tiles [4, 128, 32, 512]
    xq_b = []
    for b in range(B):
        xT = np.asarray(x[b], f32).astype(NPBF16).T  # [4096, 2048]
        xq_b.append(
            np.ascontiguousarray(
                xT.reshape(32, 128, 4, 512).transpose(2, 1, 0, 3)
            )
        )

    # per-group weight tiles
    wq_g, wk_g, wv_g, wo_g = [], [], [], []
    for g in range(4):
        wqs = np.asarray(wq[:, MQ * g : MQ * (g + 1)], f32).astype(NPBF16)
        wq_g.append(
            np.ascontiguousarray(wqs.reshape(32, 128, 8, 128).transpose(2, 1, 0, 3))
        )
        wks = np.asarray(wk[:, MKV * g : MKV * (g + 1)], f32).astype(NPBF16)
        wk_g.append(
            np.ascontiguousarray(wks.reshape(32, 128, 2, 128).transpose(2, 1, 0, 3))
        )
        wvs = np.asarray(wv[:, MKV * g : MKV * (g + 1)], f32).astype(NPBF16)
        wv_g.append(np.ascontiguousarray(wvs.reshape(32, 128, 256).transpose(1, 0, 2)))
        wos = np.asarray(wo[MQ * g : MQ * (g + 1), :], f32).astype(NPBF16)
        wo_g.append(np.ascontiguousarray(wos.reshape(8, 128, 4096).transpose(1, 0, 2)))

    in_maps = []
    for c in range(NCORES):
        b, g = c // 4, c % 4
        m = {
            "xq": xq_b[b],
            "wq": wq_g[g],
            "wk": wk_g[g],
            "wv": wv_g[g],
            "wo": wo_g[g],
            "ce": ce,
            "s2": s2,
        }
        if causal:
            m["trid"] = tri_host
            m["seld"] = sel_host
        else:
            m["maskt"] = mask_host
        in_maps.append(m)
    return in_maps


def kernel(x, start_pos, freqs_cos, freqs_sin, cache, mask, wq, wk, wv, wo):
    global LAST_EXEC_TIME_NS, LAST_RESULTS

    x = np.asarray(x)
    mask = np.asarray(mask)
    assert x.shape == (B, S, DIM), x.shape
    assert int(start_pos) == 0, "kernel specialized for start_pos=0"

    causal_ref = np.where(
        np.arange(S)[None, :] <= np.arange(S)[:, None], 0.0, NEG_INF
    ).astype(np.float32)
    causal = bool(np.array_equal(np.asarray(mask, np.float32), causal_ref))

    _install_ntff_hook()
    from concourse.bass_utils import run_bass_kernel_spmd
    import concourse.bass_utils as _bu

    trace = bool(os.environ.get("BASS_TRACE"))
    if trace:
        _bu.upload_artifacts = lambda tmpdir: tmpdir

    in_maps = _prep_inputs(x, freqs_cos, freqs_sin, mask, wq, wk, wv, wo, causal)
    nc = _build(causal)
    split_excess_waits(nc)

    res = run_bass_kernel_spmd(nc, in_maps, core_ids=list(range(NCORES)), trace=trace)
    LAST_EXEC_TIME_NS = res.exec_time_ns
    LAST_RESULTS = res

    partials = []
    for c in range(NCORES):
        # [128, 16, 4096] bf16, p-major token tiles; sum partials in f32
        o = np.asarray(res.results[c]["out"], dtype=np.float32)
        partials.append(o.transpose(1, 0, 2).reshape(S, DIM))
    out = np.stack(
        [
            partials[0] + partials[1] + partials[2] + partials[3],
            partials[4] + partials[5] + partials[6] + partials[7],
        ]
    ).astype(np.float32)
    return out


# revision 75
# speedup vs baseline: 1.0146x; 1.0146x over previous
"""Trainium2 Bass kernel for nn_Attention_54159537603130.

Dense GQA attention block (QKV proj + RoPE + causal attention + out proj),
sharded over 8 NeuronCores as (batch=2) x (kv-head groups=4).  Each core
computes a [S, DIM] partial of the output projection (wo is row-sharded);
the host sums the 4 group partials per batch.

All on-chip matmul operands live in "transposed" feature-on-partition
layouts so no large on-chip transposes are needed:
  Q^T/K^T [d, t]  -> scores^T tiles [t, s] directly
  V token-major [t, d] -> out^T = V^T @ P^T via PE accumulation
  out^T [d, s] is exactly the lhsT of the wo matmul.

Softmax runs without max-subtraction (logits are O(10) here).  The
1/sqrt(HD) scale is folded into the exp activation's scale.

Phase-2 (causal) highlights:
  - causal masking is a post-exp multiply by a shared [128,128] 0/1
    triangle (numerically identical to adding -1e9 pre-exp, off the
    scores->exp critical path, DVE 16-bit fast path);
  - diagonal tiles only process their valid column suffix, ordered so
    psum accumulate start/stop flags stay full-width;
  - scores for two i-chunks land in one [128,2,512] two-bank psum tile
    and are exponentiated by ONE ACT instruction (the ~260ns fixed ACT
    cost per instruction would otherwise dominate);
  - row sums are [128,128]-config selector matmuls (ones in column 32j)
    so the PE never switches tile config (an M=1 ones-vector rowsum
    costs ~2x95ns of reconfig per i-chunk), accumulated across a whole
    head into rows 0/32/64/96 of one psum bank; ln/exp then runs once
    per head over the full tile (ACT cost is free-size only);
  - a two-stage software pipeline (scores -> exp+mask -> PV+rowsum)
    keeps the PE fed past the ACT exp and DVE mask latency;
  - the reciprocal broadcast matmul + in-place normalize of out^T are
    deferred into the next head's stream so the single psb bank's WAR
    chain never stalls the PE.
RoPE's (2i,2i+1) partition swap runs as two stride-2 partition DMAs
instead of a PE permute matmul.  Token quarters are projected in pairs
sharing each weight-chunk DMA, with latency-critical transfers sliced
across the 8 parallel DMA queues (a whole-tile DMA only gets 1/8 of
the HBM bandwidth when other transfers are in flight).
"""

import os
import sys

sys.path.insert(0, "/opt/trn_rl_repo")

import numpy as np
import ml_dtypes

import concourse.bass as bass
import concourse.tile as tile
from concourse import mybir

BF16 = mybir.dt.bfloat16
F32 = mybir.dt.float32
NPBF16 = ml_dtypes.bfloat16

DIM, NH, NKV, HD = 4096, 32, 8, 128
B, S = 2, 2048
NCORES = 8
GQ = 8  # q heads per core
GKV = 2  # kv heads per core
MQ = GQ * HD  # 1024 q-proj cols per core
MKV = GKV * HD  # 256 kv-proj cols per core
SC = 1.0 / np.sqrt(HD)
NEG_INF = -1e9

LAST_EXEC_TIME_NS = None
LAST_RESULTS = None


def _install_ntff_hook():
    """antenv.axon_hooks is absent in this image; reconstruct the NTFF
    profiling hook via ctypes against libaxon_pjrt.so (only used when
    BASS_TRACE is set)."""
    import types
    import contextlib
    import ctypes

    if "antenv.axon_hooks" in sys.modules:
        return
    try:
        lib = ctypes.CDLL("/opt/axon/libaxon_pjrt.so")
        have = hasattr(lib, "axon_start_nrt_profile")
    except OSError:
        have = False

    if have:
        lib.axon_start_nrt_profile.argtypes = [
            ctypes.POINTER(ctypes.c_int64),
            ctypes.c_size_t,
        ]
        lib.axon_start_nrt_profile.restype = ctypes.c_int64
        lib.axon_stop_nrt_profile.argtypes = [ctypes.c_char_p]
        lib.axon_stop_nrt_profile.restype = ctypes.c_int64

        @contextlib.contextmanager
        def _hook(output_dir, device_ids):
            import jax

            jax.devices()
            if device_ids:
                ids = (ctypes.c_int64 * len(device_ids))(*device_ids)
                rc = lib.axon_start_nrt_profile(ids, len(device_ids))
            else:
                rc = lib.axon_start_nrt_profile(None, 0)
            if rc != 0:
                raise RuntimeError(f"axon_start_nrt_profile rc={rc}")
            try:
                yield
            finally:
                n = lib.axon_stop_nrt_profile(str(output_dir).encode())
                print(f"profile: {n} file(s) written to {output_dir}")

        hook = _hook
    else:
        hook = None

    mod = types.ModuleType("antenv.axon_hooks")
    mod.get_axon_ntff_profile_hook = lambda: hook
    mod.set_axon_ntff_profile_hook = lambda h: None
    sys.modules["antenv.axon_hooks"] = mod


def split_excess_waits(nc, max_waits=1):
    """walrus codegen supports very few sync waits per instruction while
    Tile's tail/release drains can carry several; hoist excess onto NOPs."""
    for fn in nc.m.functions:
        for blk in fn.blocks:
            insts = blk.instructions
            changed = False
            i = 0
            while i < len(insts):
                inst = insts[i]
                si = inst.sync_info
                if (
                    si is not None
                    and si.on_wait is not None
                    and len(si.on_wait) > max_waits
                ):
                    w = si.on_wait
                    k = 0
                    while len(w) > max_waits:
                        nop = mybir.InstNoOp(
                            name=f"{inst.name}_wsplit{k}",
                            engine=inst.engine,
                            ins=[],
                            outs=[],
                        )
                        nop.sync_info = mybir.SyncInfo(
                            on_wait=w[:max_waits], on_update=[]
                        )
                        insts.insert(i, nop)
                        i += 1
                        w = w[max_waits:]
                        k += 1
                    inst.sync_info = mybir.SyncInfo(on_wait=w, on_update=si.on_update)
                    changed = True
                i += 1
            if changed:
                blk.instructions = insts


def _build(causal: bool):
    nc = bass.Bass("TRN2", target_bir_lowering=False, debug=False)
    Exp = mybir.ActivationFunctionType.Exp
    Ln = mybir.ActivationFunctionType.Ln

    # DRAM I/O — all inputs pre-tiled on the host into SBUF-friendly
    # [partition, ...] layouts with large contiguous per-partition runs.
    xq_d = nc.dram_tensor("xq", [4, 128, 32, 512], BF16, kind="ExternalInput").ap()
    wq_d = nc.dram_tensor("wq", [8, 128, 32, 128], BF16, kind="ExternalInput").ap()
    wk_d = nc.dram_tensor("wk", [2, 128, 32, 128], BF16, kind="ExternalInput").ap()
    wv_d = nc.dram_tensor("wv", [128, 32, 256], BF16, kind="ExternalInput").ap()
    wo_d = nc.dram_tensor("wo", [128, 8, 4096], BF16, kind="ExternalInput").ap()
    ce_d = nc.dram_tensor("ce", [128, 2048], BF16, kind="ExternalInput").ap()
    s2_d = nc.dram_tensor("s2", [128, 2048], BF16, kind="ExternalInput").ap()
    sel_d = nc.dram_tensor("seld", [128, 4, 128], BF16, kind="ExternalInput").ap()
    if causal:
        # one shared [128,128] 0/1 triangle masks every diagonal tile
        tri_d = nc.dram_tensor("trid", [128, 128], BF16, kind="ExternalInput").ap()
    else:
        mask_d = nc.dram_tensor(
            "maskt", [128, 16, 2048], BF16, kind="ExternalInput"
        ).ap()
    out_d = nc.dram_tensor("out", [128, 16, 4096], BF16, kind="ExternalOutput").ap()

    with tile.TileContext(nc) as tc:
        with (
            tc.tile_pool(name="consts", bufs=1) as consts,
            tc.tile_pool(name="persist", bufs=1) as persist,
        ):
            # tiles declared up front; DMA emission ordered so the first
            # projection's deps (x quarter 0, wq chunk 0) load first.
            ce_t = consts.tile([128, 2048], BF16)
            s2_t = consts.tile([128, 2048], BF16)
            sel_t = consts.tile([128, 4, 128], BF16)
            ones_col = consts.tile([128, 1], BF16)
            ones_row = consts.tile([1, 128], BF16)
            ones128 = consts.tile([128, 128], BF16)

            qrot = persist.tile([128, GQ, 2048], BF16)
            krot = persist.tile([128, GKV, 2048], BF16)
            vtok = persist.tile([128, 16, MKV], BF16)
            if causal:
                tri_t = persist.tile([128, 128], BF16)

            # ---------------- Phase 1: QKV projections + RoPE --------------
            with (
                tc.tile_pool(name="p1", bufs=1) as p1,
                tc.tile_pool(name="p1ps", bufs=1, space="PSUM") as pps,
            ):
                wv_t = p1.tile([128, 32, 256], BF16, tag="wv", bufs=1)

                rope_tail = []

                def rope(ps, dst, toff, defer=False):
                    # dst = ce*q + s2*pairswap(q), all [128, 512] at t-offset.
                    # The (2i,2i+1) partition swap runs as two stride-2
                    # partition DMAs (sbuf->sbuf) instead of a PE permute
                    # matmul, and the multiplies stay all-bf16-SBUF for the
                    # DVE 2x mode.  Only the psum->sbuf copy is immediate
                    # (it frees the psum slot); during the DMA-critical
                    # first token pair the swap DMAs are deferred behind
                    # the next weight chunk's DMA so they never delay it.
                    qb = p1.tile([128, 512], BF16, tag="ropeb", bufs=5)
                    nc.scalar.copy(out=qb, in_=ps)

                    def tail():
                        sw = p1.tile([128, 512], BF16, tag="swap", bufs=3)
                        nc.gpsimd.dma_start(out=sw[0:127:2, :], in_=qb[1:128:2, :])
                        nc.gpsimd.dma_start(out=sw[1:128:2, :], in_=qb[0:127:2, :])
                        a = p1.tile([128, 512], BF16, tag="ropea", bufs=3)
                        nc.vector.tensor_mul(a, qb, ce_t[:, toff : toff + 512])
                        bt = p1.tile([128, 512], BF16, tag="ropec", bufs=3)
                        nc.vector.tensor_mul(bt, sw, s2_t[:, toff : toff + 512])
                        nc.vector.tensor_add(dst, a, bt)

                    if defer:
                        rope_tail.append(tail)
                    else:
                        tail()

                def flush_ropes():
                    while rope_tail:
                        rope_tail.pop(0)()

                # token quarters processed in pairs sharing each weight
                # chunk DMA (halves the wq/wk streaming traffic, which
                # otherwise saturates the DMA queue early in the kernel)
                xh_t = {}

                def load_xh(q):
                    xh = p1.tile([128, 32, 512], BF16, tag="xh", bufs=3, name="xh")
                    xh_t[q] = xh
                    if q == 0:
                        # interleave fine-grained slices of the first x/wq
                        # tiles so the first matmuls start ~4MB sooner, with
                        # the rope constants right behind
                        # transfers complete in issue order at the full
                        # aggregate DMA rate (packets spray across all 16
                        # engines), so issue in DEMAND order: the first
                        # matmul starts once wqc0's and xh's first slices
                        # land and then races the rest of the stream
                        nc.gpsimd.dma_start(out=wqc0[:, 0:16], in_=wq_d[0, :, 0:16])
                        nc.gpsimd.dma_start(out=xh[:, 0:8], in_=xq_d[q, :, 0:8])
                        nc.gpsimd.dma_start(out=xh[:, 8:16], in_=xq_d[q, :, 8:16])
                        nc.gpsimd.dma_start(out=wqc0[:, 16:32], in_=wq_d[0, :, 16:32])
                        nc.gpsimd.dma_start(out=xh[:, 16:24], in_=xq_d[q, :, 16:24])
                        nc.gpsimd.dma_start(out=xh[:, 24:32], in_=xq_d[q, :, 24:32])
                        nc.vector.memset(ones_col, 1.0)
                        nc.vector.memset(ones_row, 1.0)
                        nc.vector.memset(ones128, 1.0)
                    elif q == 1:
                        for c in range(0, 32, 8):
                            nc.gpsimd.dma_start(
                                out=xh[:, c : c + 8], in_=xq_d[q, :, c : c + 8]
                            )
                    else:
                        nc.gpsimd.dma_start(out=xh[:, 0:16], in_=xq_d[q, :, 0:16])
                        nc.gpsimd.dma_start(out=xh[:, 16:32], in_=xq_d[q, :, 16:32])

                wqc0 = p1.tile([128, 32, 128], BF16, tag="wc", bufs=3)
                load_xh(0)
                load_xh(1)
                # rope constants ride behind x quarter 1: they are only
                # needed by the off-critical-path rope DVE multiplies
                nc.gpsimd.dma_start(out=ce_t, in_=ce_d)
                nc.gpsimd.dma_start(out=s2_t, in_=s2_d)

                for qp in (0, 2):
                    if qp == 2:
                        load_xh(3)
                        if causal:
                            nc.gpsimd.dma_start(out=tri_t, in_=tri_d)
                            nc.gpsimd.dma_start(out=sel_t, in_=sel_d)
                    for m in range(GQ):
                        if qp == 0 and m == 0:
                            wqc = wqc0
                        else:
                            wqc = p1.tile([128, 32, 128], BF16, tag="wc", bufs=3)
                            nc.gpsimd.dma_start(
                                out=wqc[:, 0:16], in_=wq_d[m, :, 0:16]
                            )
                            nc.gpsimd.dma_start(
                                out=wqc[:, 16:32], in_=wq_d[m, :, 16:32]
                            )
                        if qp == 0 and m == 2:
                            nc.gpsimd.dma_start(out=wv_t, in_=wv_d)
                        if qp == 0 and m == 5:
                            load_xh(2)
                        flush_ropes()
                        for q in (qp, qp + 1):
                            t0 = 512 * q
                            ps = pps.tile([128, 512], F32, tag="proj", bufs=3)
                            for d in range(32):
                                nc.tensor.matmul(
                                    out=ps,
                                    lhsT=wqc[:, d],
                                    rhs=xh_t[q][:, d],
                                    start=(d == 0),
                                    stop=(d == 31),
                                )
                            rope(ps, qrot[:, m, t0 : t0 + 512], t0,
                                 defer=(qp == 0 and m < 6))
                    for m in range(GKV):
                        wkc = p1.tile([128, 32, 128], BF16, tag="wc", bufs=3)
                        nc.gpsimd.dma_start(out=wkc[:, 0:16], in_=wk_d[m, :, 0:16])
                        nc.gpsimd.dma_start(out=wkc[:, 16:32], in_=wk_d[m, :, 16:32])
                        flush_ropes()
                        for q in (qp, qp + 1):
                            t0 = 512 * q
                            ps = pps.tile([128, 512], F32, tag="proj", bufs=3)
                            for d in range(32):
                                nc.tensor.matmul(
                                    out=ps,
                                    lhsT=wkc[:, d],
                                    rhs=xh_t[q][:, d],
                                    start=(d == 0),
                                    stop=(d == 31),
                                )
                            rope(ps, krot[:, m, t0 : t0 + 512], t0)
                    for q in (qp, qp + 1):
                        for tv in range(4):
                            psv = pps.tile([128, 256], F32, tag="vproj", bufs=3)
                            for d in range(32):
                                nc.tensor.matmul(
                                    out=psv,
                                    lhsT=xh_t[q][:, d, 128 * tv : 128 * tv + 128],
                                    rhs=wv_t[:, d],
                                    start=(d == 0),
                                    stop=(d == 31),
                                )
                            nc.scalar.copy(out=vtok[:, 4 * q + tv, :], in_=psv)

            # outT lives from phase 2 through phase 3; allocated after the
            # phase-1 pools release so SBUF peaks stay under budget. wo is
            # prefetched here so its 8MB load overlaps phase 2.
            outT_pool = tc.alloc_tile_pool(name="po", bufs=1)
            outT = outT_pool.tile([128, GQ, 2048], BF16)
            wo_pool = tc.alloc_tile_pool(name="pwo", bufs=1)
            wo_t = wo_pool.tile([128, 8, 4096], BF16)
            nc.gpsimd.dma_start(out=wo_t, in_=wo_d)

            # ---------------- Phase 2: attention ---------------------------
            with (
                tc.tile_pool(name="p2", bufs=1) as p2,
                tc.tile_pool(name="p2ps", bufs=1, space="PSUM") as pps2,
            ):
                if causal:
                    # Two-stage software pipeline: scores run LAG1 items
                    # ahead of exp+mask, which run one more item ahead of
                    # PV+rowsum, so neither the ACT exp latency nor the DVE
                    # mask multiply ever gates the PE.
                    # PSUM: pss 3 + pso 2 + rs 2 + psb 1 = 8 banks.
                    # Diagonal tiles of the j>=1 blocks only process their
                    # valid column suffix [c0:512); they are ordered first
                    # in each block so the start flag (p=0, full width) and
                    # stop flag (last non-diag, full width) cover the whole
                    # tile.  Rowsums use a [128,128] selector lhsT (ones in
                    # column 32j) so every phase-2 matmul runs in the same
                    # 128x128 PE tile config — an M=1 ones-vector rowsum
                    # switches the PE to a 128x32 config and back, costing
                    # ~95ns twice per i-chunk.  One accumulate group spans
                    # the whole head (start wipes the bank at head start;
                    # other rows just accumulate selector zeros), and the
                    # banks alternate per head parity so the next head's
                    # rowsums never wait on this head's ln read.
                    rs_a = pps2.tile([128, 512], F32, name="rs_a")
                    rs_par = [rs_a, rs_a]
                    LAG1 = 1

                    # i-chunk pairs: one two-bank psum tile and ONE exp
                    # instruction per pair (the ~260ns fixed ACT cost per
                    # instruction is a big slice of phase 2).  Diagonal
                    # tiles are interleaved with full-width ones so the
                    # paired exp's [cmin:512) window wastes few columns.
                    pairs = []  # item = (h,j,i,c0,first,last,rs_start,rs_stop)
                    for h in range(GQ):
                        for bi, j in enumerate(range(4)):
                            if j == 0:
                                ilist = [(i, 128 * i) for i in range(4)]
                            else:
                                dg = [(4 * j + p, 128 * p) for p in range(4)]
                                nd = [(i, 0) for i in range(4 * j)]
                                ilist = [dg[0], nd[0], dg[1], dg[2], dg[3], nd[1]]
                                ilist += nd[2:]
                            n = len(ilist)
                            items = [
                                (h, j, i, c0, idx == 0, idx == n - 1,
                                 bi == 0 and idx == 0, bi == 3 and idx == n - 1)
                                for idx, (i, c0) in enumerate(ilist)
                            ]
                            for t in range(0, len(items), 2):
                                pairs.append((items[t], items[t + 1]))

                    def emit_scores(pair):
                        pss = pps2.tile([128, 2, 512], F32, tag="pss", bufs=2)
                        for half, item in enumerate(pair):
                            h, j, i, c0, first, last, rs_s, rs_e = item
                            kv = h // 4
                            s0 = 512 * j
                            nc.tensor.matmul(
                                out=pss[:, half, c0:512],
                                lhsT=krot[:, kv, 128 * i : 128 * i + 128],
                                rhs=qrot[:, h, s0 + c0 : s0 + 512],
                                start=True,
                                stop=True,
                            )
                        return pss

                    def emit_expmask(pair, pss):
                        cmin = min(pair[0][3], pair[1][3])
                        e = p2.tile([128, 2, 512], BF16, tag="exp", bufs=3)
                        nc.scalar.activation(
                            out=e[:, :, cmin:512],
                            in_=pss[:, :, cmin:512],
                            func=Exp,
                            scale=SC,
                        )
                        for half, item in enumerate(pair):
                            h, j, i, c0, first, last, rs_s, rs_e = item
                            if i >= 4 * j:
                                sl = e[:, half, c0 : c0 + 128]
                                nc.vector.tensor_mul(sl, sl, tri_t)
                        return e

                    def emit_pvrs(pair, e):
                        for half, item in enumerate(pair):
                            h, j, i, c0, first, last, rs_s, rs_e = item
                            kv = h // 4
                            s0 = 512 * j
                            rs_t = rs_par[h % 2]
                            pso = pso_cur[0]
                            nc.tensor.matmul(
                                out=pso[:, c0:512],
                                lhsT=vtok[:, i, 128 * kv : 128 * kv + 128],
                                rhs=e[:, half, c0:512],
                                start=first,
                                stop=last,
                            )
                            nc.tensor.matmul(
                                out=rs_t[:, c0:512],
                                lhsT=sel_t[:, j, :],
                                rhs=e[:, half, c0:512],
                                start=rs_s,
                                stop=rs_e,
                            )
                            if last:
                                # block done: park the raw out^T
                                nc.vector.tensor_copy(
                                    out=outT[:, h, s0 : s0 + 512], in_=pso
                                )
                                pso_cur[0] = pps2.tile(
                                    [128, 512], F32, tag="pso", bufs=2, name="pso_n"
                                )
                                if rs_e:
                                    finalize_h(h)

                    def finalize_h(h):
                        # batched softmax denominators for this head's 4
                        # blocks (rows 0/32/64/96), computed on the DVE
                        # (idle here) instead of ACT ln+exp: the ACT queue
                        # sits 2+ paired exps deep at head boundaries, so
                        # an ACT-produced recip stalls the psb matmul.
                        # The per-j psum broadcast + in-place multiply are
                        # deferred >=2 steps into the next head's stream so
                        # neither the recip nor the single psb bank's WAR
                        # chain ever gates the PE.
                        lnr = p2.tile([128, 512], F32, tag="lnr", bufs=2)
                        nc.scalar.activation(out=lnr, in_=rs_par[h % 2], func=Ln)
                        recip = p2.tile([128, 512], BF16, tag="recip", bufs=2)
                        nc.scalar.activation(out=recip, in_=lnr, func=Exp, scale=-1.0)
                        for j in range(4):
                            deferred.append((step2[0], h, j, recip))

                    def run_deferred(drain=False):
                        rdy, h, j, recip = deferred.pop(0)
                        if drain and j % 2 == 1:
                            pbig = pps2.tile(
                                [128, 2, 512], F32, tag="pss", bufs=2, name="psbd"
                            )
                            psb = pbig[:, 0, :]
                        else:
                            psb = pps2.tile([128, 512], F32, tag="psb", bufs=1)
                        nc.tensor.matmul(
                            out=psb,
                            lhsT=ones128[32 * j : 32 * j + 1, :],
                            rhs=recip[32 * j : 32 * j + 1, :],
                            start=True,
                            stop=True,
                            tile_position=(32 * j, 0),
                        )
                        sl = outT[:, h, 512 * j : 512 * j + 512]
                        nc.vector.tensor_mul(sl, sl, psb)

                    pso0 = pps2.tile([128, 512], F32, tag="pso", bufs=2, name="pso_n")
                    pso_cur = [pso0]
                    step2 = [0]
                    deferred = []
                    q_exp = []
                    q_pv = []
                    for k in range(len(pairs) + LAG1 + 1):
                        if k < len(pairs):
                            q_exp.append((pairs[k], emit_scores(pairs[k])))
                        if k >= LAG1 and q_exp:
                            pr, pss = q_exp.pop(0)
                            q_pv.append((pr, emit_expmask(pr, pss)))
                        if k >= LAG1 + 1 and q_pv:
                            pr, e = q_pv.pop(0)
                            step2[0] += 1
                            emit_pvrs(pr, e)
                            if deferred and deferred[0][0] <= step2[0]:
                                run_deferred()
                    while deferred:
                        run_deferred(drain=True)
                else:
                    # generic-mask fallback: original structure (mask added
                    # pre-exp, per-block rowsum + broadcast matmul), j-outer
                    # so the mask streams in [128, 16, 512] column slices.
                    def finalize(fin):
                        pso_, psr_, h_, s0_ = fin
                        nc.scalar.activation(out=psr_, in_=psr_, func=Ln)
                        rp = p2.tile([1, 512], BF16, tag="rp", bufs=2)
                        nc.scalar.activation(out=rp, in_=psr_, func=Exp, scale=-1.0)
                        psb = pps2.tile([128, 512], F32, tag="psb", bufs=1)
                        nc.tensor.matmul(
                            out=psb, lhsT=ones_row, rhs=rp, start=True, stop=True
                        )
                        rb = p2.tile([128, 512], F32, tag="rb", bufs=2)
                        nc.vector.tensor_copy(out=rb, in_=psb)
                        nc.vector.tensor_mul(outT[:, h_, s0_ : s0_ + 512], pso_, rb)

                    pending = None
                    for j in range(4):
                        s0 = 512 * j
                        mask_j = p2.tile([128, 16, 512], BF16, tag="mzk", bufs=2)
                        nc.gpsimd.dma_start(
                            out=mask_j, in_=mask_d[:, :, s0 : s0 + 512]
                        )
                        for h in range(GQ):
                            kv = h // 4
                            n_i = 16
                            pso = pps2.tile([128, 512], F32, tag="pso", bufs=2)
                            psr = pps2.tile([1, 512], F32, tag="psr", bufs=2)
                            for idx in range(n_i):
                                pss = pps2.tile([128, 512], F32, tag="pss", bufs=3)
                                nc.tensor.matmul(
                                    out=pss,
                                    lhsT=krot[:, kv, 128 * idx : 128 * idx + 128],
                                    rhs=qrot[:, h, s0 : s0 + 512],
                                    start=True,
                                    stop=True,
                                )
                                nc.vector.tensor_add(pss, pss, mask_j[:, idx, :])
                                e = p2.tile([128, 512], BF16, tag="exp", bufs=6)
                                nc.scalar.activation(out=e, in_=pss, func=Exp, scale=SC)
                                nc.tensor.matmul(
                                    out=pso,
                                    lhsT=vtok[:, idx, 128 * kv : 128 * kv + 128],
                                    rhs=e,
                                    start=(idx == 0),
                                    stop=(idx == n_i - 1),
                                )
                                nc.tensor.matmul(
                                    out=psr[0:1, :],
                                    lhsT=ones_col,
                                    rhs=e,
                                    start=(idx == 0),
                                    stop=(idx == n_i - 1),
                                )
                                if idx == 0 and pending is not None:
                                    finalize(pending)
                                    pending = None
                            if pending is not None:
                                finalize(pending)
                            pending = (pso, psr, h, s0)
                    finalize(pending)

            # ---------------- Phase 3: output projection -------------------
            with (
                tc.tile_pool(name="p3", bufs=1) as p3,
                tc.tile_pool(name="p3ps", bufs=1, space="PSUM") as pps3,
            ):
                # m-outer ordering: one LDWEIGHTS per 8 matmuls (all 8 PSUM
                # banks accumulate in parallel across the dc dimension)
                for s in range(16):
                    psfs = [
                        pps3.tile(
                            [128, 512], F32, tag="psf", bufs=8, name=f"psf_{s}_{dc}"
                        )
                        for dc in range(8)
                    ]
                    for m in range(8):
                        for dc in range(8):
                            nc.tensor.matmul(
                                out=psfs[dc],
                                lhsT=outT[:, m, 128 * s : 128 * s + 128],
                                rhs=wo_t[:, m, 512 * dc : 512 * dc + 512],
                                start=(m == 0),
                                stop=(m == 7),
                            )
                    for dc in range(8):
                        ot = p3.tile([128, 512], BF16, tag="ot", bufs=16)
                        if dc % 2 == 0:
                            nc.scalar.copy(out=ot, in_=psfs[dc])
                        else:
                            nc.vector.tensor_copy(out=ot, in_=psfs[dc])
                        nc.gpsimd.dma_start(
                            out=out_d[:, s, 512 * dc : 512 * dc + 512], in_=ot
                        )

            wo_pool.release()
            outT_pool.release()

    return nc


def _prep_inputs(x, freqs_cos, freqs_sin, mask, wq, wk, wv, wo, causal):
    """Host-side shard + retile into the DMA layouts declared in _build."""
    f32 = np.float32

    # RoPE planes [128, 2048]: ce[2i,t]=ce[2i+1,t]=cos[t,i];
    # s2[2i,t]=-sin[t,i], s2[2i+1,t]=+sin[t,i].  (1/sqrt(HD) now lives in
    # the exp activation scale, so q and k share one pair of planes.)
    cos_t = np.asarray(freqs_cos, f32).T  # [64, 2048]
    sin_t = np.asarray(freqs_sin, f32).T
    ce = np.repeat(cos_t, 2, axis=0).astype(NPBF16)  # [128, 2048]
    s2 = np.empty((HD, S), f32)
    s2[0::2] = -sin_t
    s2[1::2] = sin_t
    s2 = s2.astype(NPBF16)

    maskT = np.ascontiguousarray(np.asarray(mask, f32).T)  # [t, s]
    if causal:
        tri_host = (
            np.arange(128)[:, None] <= np.arange(128)[None, :]
        ).astype(NPBF16)
        sel_host = np.zeros((128, 4, 128), NPBF16)
        for j in range(4):
            sel_host[:, j, 32 * j] = 1
    else:
        # additive mask on UNSCALED scores: pre-divide by SC
        mask_host = np.ascontiguousarray(
            (maskT / SC).reshape(16, 128, 2048).transpose(1, 0, 2)
        ).astype(NPBF16)

    # per-batch x tiles [4, 128, 32, 512]
    xq_b = []
    for b in range(B):
        xT = np.asarray(x[b], f32).astype(NPBF16).T  # [4096, 2048]
        xq_b.append(
            np.ascontiguousarray(
                xT.reshape(32, 128, 4, 512).transpose(2, 1, 0, 3)
            )
        )

    # per-group weight tiles
    wq_g, wk_g, wv_g, wo_g = [], [], [], []
    for g in range(4):
        wqs = np.asarray(wq[:, MQ * g : MQ * (g + 1)], f32).astype(NPBF16)
        wq_g.append(
            np.ascontiguousarray(wqs.reshape(32, 128, 8, 128).transpose(2, 1, 0, 3))
        )
        wks = np.asarray(wk[:, MKV * g : MKV * (g + 1)], f32).astype(NPBF16)
        wk_g.append(
            np.ascontiguousarray(wks.reshape(32, 128, 2, 128).transpose(2, 1, 0, 3))
        )
        wvs = np.asarray(wv[:, MKV * g : MKV * (g + 1)], f32).astype(NPBF16)
        wv_g.append(np.ascontiguousarray(wvs.reshape(32, 128, 256).transpose(1, 0, 2)))
        wos = np.asarray(wo[MQ * g : MQ * (g + 1), :], f32).astype(NPBF16)
        wo_g.append(np.ascontiguousarray(wos.reshape(8, 128, 4096).transpose(1, 0, 2)))

    in_maps = []
    for c in range(NCORES):
        b, g = c // 4, c % 4
        m = {
            "xq": xq_b[b],
            "wq": wq_g[g],
            "wk": wk_g[g],
            "wv": wv_g[g],
            "wo": wo_g[g],
            "ce": ce,
            "s2": s2,
        }
        if causal:
            m["trid"] = tri_host
            m["seld"] = sel_host
        else:
            m["maskt"] = mask_host
        in_maps.append(m)
    return in_maps


def kernel(x, start_pos, freqs_cos, freqs_sin, cache, mask, wq, wk, wv, wo):
    global LAST_EXEC_TIME_NS, LAST_RESULTS

    x = np.asarray(x)
    mask = np.asarray(mask)
    assert x.shape == (B, S, DIM), x.shape
    assert int(start_pos) == 0, "kernel specialized for start_pos=0"

    causal_ref = np.where(
        np.arange(S)[None, :] <= np.arange(S)[:, None], 0.0, NEG_INF
    ).astype(np.float32)
    causal = bool(np.array_equal(np.asarray(mask, np.float32), causal_ref))

    _install_ntff_hook()
    from concourse.bass_utils import run_bass_kernel_spmd
    import concourse.bass_utils as _bu

    trace = bool(os.environ.get("BASS_TRACE"))
    if trace:
        _bu.upload_artifacts = lambda tmpdir: tmpdir

    in_maps = _prep_inputs(x, freqs_cos, freqs_sin, mask, wq, wk, wv, wo, causal)
    nc = _build(causal)
    split_excess_waits(nc)

    res = run_bass_kernel_spmd(nc, in_maps, core_ids=list(range(NCORES)), trace=trace)
    LAST_EXEC_TIME_NS = res.exec_time_ns
    LAST_RESULTS = res

    partials = []
    for c in range(NCORES):
        # [128, 16, 4096] bf16, p-major token tiles; sum partials in f32
        o = np.asarray(res.results[c]["out"], dtype=np.float32)
        partials.append(o.transpose(1, 0, 2).reshape(S, DIM))
    out = np.stack(
        [
            partials[0] + partials[1] + partials[2] + partials[3],
            partials[4] + partials[5] + partials[6] + partials[7],
        ]
    ).astype(np.float32)
    return out
